# revision 10
# baseline (speedup 1.0000x reference)
"""M3GNet multi-task head kernel for 8 Trainium2 NeuronCores.

Math restructuring (exactly equivalent to the reference up to fp reassociation):
  logits_i = x_i @ v + c0,  v = w_proj @ w_att, c0 = b_proj@w_att + b_att.
  softmax over all nodes: alpha_i = exp(logits_i)/Z  (c0 and the max-subtraction
  cancel in the ratio; |x@v| < ~0.3 so exp is numerically safe).
  pooled[g] = sum_{i in g} alpha_i * h_i
            = ( (sum_{i in g} w_i x_i) @ w_proj + (sum_{i in g} w_i) b_proj ) / Z
  with w_i = exp(x_i @ v).  So the device only needs weighted segment sums of x
  (u[g] in R^64) plus the weight sums W[g]; h is never materialized.
  Head layer 1:  relu(pooled @ w1_top + sg @ w1_bot + b1)
     pooled @ w1_top = (1/Z) * uhat @ (w_proj @ w1_top  stacked with b_proj@w1_top)
  so per head we precompute A_h = [[w_proj @ w1_top_h], [b_proj @ w1_top_h]] (65x128).

Distribution: graph-aligned data parallel.  Core s owns graphs
[s*G/8, (s+1)*G/8); since `batch` is sorted its nodes form a contiguous range.
Only the scalar Z = sum_i w_i needs an AllReduce.

SPMD uniformity: one program runs on all 8 cores, so no instruction operand may
depend on per-core data.  All data-dependent structure is carried in DMA'd
mask *content*:
  Level 1: each 128-node chunk is reduced to <=R "slots" via a narrow one-hot
    (R = max graphs spanned by any chunk).  Slot placement inside the psum tile
    is by *chunk index* (uniform): chunk c gets columns R*idx(c) of a zero-padded
    [128, S] lhsT slab, matmul out = psum[32*sub : 32*sub+S] (base in {0,32,64}).
  Level 2: a [96, WIN] 0/1 matrix (host-built per core) maps the 96 slot rows of
    each psum tile to the 128 graphs of the owning window(s); accumulated into a
    per-window [WIN, 65] psum tile at base 0.  All offsets uniform.
Host pads every window (128 consecutive graphs) to a common node count so all
loop trip counts are identical across cores.
"""

import contextlib

import numpy as np

import concourse.bass as bass
import concourse.bacc as bacc
import concourse.tile as tile
from concourse import mybir
from concourse import bass_utils

NCORES = 8
F = 64          # x feature dim
FA = F + 1      # + ones column (gives the weight-sums W[g] for free)
H = 128         # hidden
OD = (64, 3, 7, 3)            # head output dims (energy, stability, crystal, material)
ODOFF = (0, 64, 67, 74)
OSUM = 77
SUBS = 3        # psum sub-blocks per level-1 tile (bases 0/32/64)

FP = mybir.dt.float32


# --------------------------------------------------------------------------
# planning (host, data-dependent but shape-uniform across cores)
# --------------------------------------------------------------------------
class Plan:
    pass


def _make_plan(batch, n_nodes, n_graphs):
    p = Plan()
    p.n_nodes = n_nodes
    p.n_graphs = n_graphs
    p.gpc = n_graphs // NCORES                     # graphs per core
    p.WIN = min(128, p.gpc)                        # graphs per window
    p.NWPC = p.gpc // p.WIN                        # windows per core
    nwg = NCORES * p.NWPC                          # global window count
    ws = np.searchsorted(batch, np.arange(nwg) * p.WIN, side="left")
    we = np.searchsorted(batch, (np.arange(nwg) + 1) * p.WIN, side="left")
    p.wstart, p.wend = ws, we
    wcnt = we - ws

    def spans_for(nch_w):
        nch = p.NWPC * nch_w
        a = np.zeros((NCORES, nch), np.int32)
        span = np.zeros((NCORES, nch), np.int32)
        for s in range(NCORES):
            for v in range(p.NWPC):
                wg = s * p.NWPC + v
                b = batch[ws[wg]:we[wg]] - (wg * p.WIN)
                nreal = len(b)
                for c in range(min(nch_w, -(-nreal // 128))):
                    lo, hi = c * 128, min((c + 1) * 128, nreal)
                    cc = v * nch_w + c
                    a[s, cc] = b[lo]
                    span[s, cc] = b[hi - 1] - b[lo] + 1
        return a, span

    # spans don't depend on tail padding; probe with minimal chunk count
    nch_w_raw = max(1, -(-int(wcnt.max()) // 128))
    _, span0 = spans_for(nch_w_raw)
    rmax = max(1, int(span0.max()))
    assert rmax <= 32, f"chunk spans {rmax} graphs > 32; layout assumption broken"
    p.R = 1 << (rmax - 1).bit_length()             # power of two -> divides 32
    p.CPS = 32 // p.R                              # chunks per 32-row sub-block
    p.S = 32                                       # lhsT slab width
    # pick sub-blocks per psum tile (1..3) minimizing window padding
    p.SUBS = min((-(-nch_w_raw // (p.CPS * s)) * (p.CPS * s), -s)
                 for s in (1, 2, 3))[1] * -1
    p.CPT = p.CPS * p.SUBS                         # chunks per level-1 psum tile
    p.P2R = 32 * p.SUBS                            # slot rows per psum tile
    # pad windows to whole level-1 psum tiles: no tile straddles a window
    p.NCH_W = -(-nch_w_raw // p.CPT) * p.CPT
    p.NWN = p.NCH_W * 128
    p.NCH = p.NWPC * p.NCH_W
    p.a, p.span = spans_for(p.NCH_W)
    p.NB2 = p.NCH // p.CPT                         # level-2 K-chunks (psum tiles)
    p.TPW = p.NCH_W // p.CPT                       # tiles per window
    p.key = (p.n_nodes, p.n_graphs, p.NWN, p.R, p.NWPC, p.WIN, p.SUBS)
    return p


# --------------------------------------------------------------------------
# device program
# --------------------------------------------------------------------------
def _build(p):
    nc = bacc.Bacc("TRN2", target_bir_lowering=False, debug=False,
                   num_devices=NCORES)
    WIN, NWPC, NCH_W, NCH = p.WIN, p.NWPC, p.NCH_W, p.NCH
    R, S, CPS, CPT, NB2 = p.R, p.S, p.CPS, p.CPT, p.NB2
    gpc = p.gpc

    xr = nc.dram_tensor("xr", [128, NCH * FA], FP, kind="ExternalInput").ap()
    m1 = nc.dram_tensor("m1", [128, NCH * R], FP, kind="ExternalInput").ap()
    m2 = nc.dram_tensor("m2", [p.P2R, NB2 * WIN], FP, kind="ExternalInput").ap()
    sgt = nc.dram_tensor("sgt", [H, gpc], FP, kind="ExternalInput").ap()
    vrep = nc.dram_tensor("vrep", [128, F], FP, kind="ExternalInput").ap()
    ident = nc.dram_tensor("ident", [128, 128], FP, kind="ExternalInput").ap()
    aab = nc.dram_tensor("aab", [FA, 4 * H], FP, kind="ExternalInput").ap()
    w1b = nc.dram_tensor("w1b", [H, 4 * H], FP, kind="ExternalInput").ap()
    w2p = nc.dram_tensor("w2p", [H, 4 * OSUM], FP, kind="ExternalInput").ap()
    b1 = nc.dram_tensor("b1", [H, 4], FP, kind="ExternalInput").ap()
    b2 = nc.dram_tensor("b2", [OSUM, 1], FP, kind="ExternalInput").ap()
    out = nc.dram_tensor("out", [OSUM, gpc], FP, kind="ExternalOutput").ap()

    xr3 = xr.rearrange("p (c f) -> p c f", c=NCH)
    m13 = m1.rearrange("p (c r) -> p c r", c=NCH)
    SL = 32                                    # logits slice width (chunks)

    with tile.TileContext(nc) as tc, contextlib.ExitStack() as ctx:
        const = ctx.enter_context(tc.tile_pool(name="const", bufs=1))
        sb = ctx.enter_context(tc.tile_pool(name="sb", bufs=2))
        sbw = ctx.enter_context(tc.tile_pool(name="sbw", bufs=3))
        persist = ctx.enter_context(tc.tile_pool(name="persist", bufs=1))

        vrep_t = const.tile([128, F], FP)
        nc.sync.dma_start(vrep_t[:], vrep[:])
        ident_t = const.tile([128, 128], FP)
        nc.sync.dma_start(ident_t[:], ident[:])
        sgt_t = const.tile([H, gpc], FP)
        nc.sync.dma_start(sgt_t[:], sgt[:])
        aab_t = const.tile([FA, 4 * H], FP)
        nc.sync.dma_start(aab_t[:], aab[:])
        w1b_t = const.tile([H, 4 * H], FP)
        nc.sync.dma_start(w1b_t[:], w1b[:])
        w2p_t = const.tile([H, 4 * OSUM], FP)
        nc.sync.dma_start(w2p_t[:], w2p[:])
        b1_t = const.tile([H, 4], FP)
        nc.sync.dma_start(b1_t[:], b1[:])
        b2_t = const.tile([OSUM, 1], FP)
        nc.sync.dma_start(b2_t[:], b2[:])

        uts = persist.tile([128, NWPC * FA], FP)     # per-window uT results
        uu = persist.tile([FA, gpc], FP)             # transposed (u | W) rows

        with tc.tile_pool(name="ps1", bufs=4, space="PSUM") as ps1pool, \
             tc.tile_pool(name="psw", bufs=3, space="PSUM") as pswpool:
            for v in range(NWPC):
                c0 = v * NCH_W
                xt = sbw.tile([128, NCH_W, FA], FP, tag="xt")
                nc.sync.dma_start(xt[:], xr3[:, c0:c0 + NCH_W, :])
                m1t = sbw.tile([128, NCH_W, R], FP, tag="m1t")
                nc.sync.dma_start(m1t[:], m13[:, c0:c0 + NCH_W, :])
                m2w = sbw.tile([p.P2R, p.TPW * WIN], FP, tag="m2w")
                nc.sync.dma_start(m2w[:], m2[:, v * p.TPW * WIN:(v + 1) * p.TPW * WIN])

                # logits -> exp weights, in SL-chunk slices
                lg = sb.tile([128, NCH_W], FP, tag="lg")
                for k in range(0, NCH_W, SL):
                    kn = min(SL, NCH_W - k)
                    prod = sb.tile([128, SL, F], FP, tag="prod", name=f"prod_{v}_{k}")
                    vb = bass.AP(tensor=vrep_t.tensor, offset=vrep_t.offset,
                                 ap=[vrep_t.ap[0], [0, kn], [1, F]])
                    nc.vector.tensor_mul(prod[:, :kn, :], xt[:, k:k + kn, 0:F], vb)
                    nc.vector.reduce_sum(lg[:, k:k + kn], prod[:, :kn, :],
                                         axis=mybir.AxisListType.X)
                wv = sb.tile([128, NCH_W], FP, tag="wv")
                nc.scalar.activation(wv[:], lg[:], mybir.ActivationFunctionType.Exp)

                # zero-padded slot slabs:  woh[p, c, R*idx(c)+r] = m1*w
                woh = sbw.tile([128, NCH_W, S], FP, tag="woh")
                nc.gpsimd.memset(woh[:], 0.0)
                psw = pswpool.tile([WIN, FA], FP, tag="psw", name=f"psw_{v}")
                for tl in range(p.TPW):                 # level-1 psum tiles
                    t = v * p.TPW + tl
                    lo = tl * CPT                       # chunk offset in window
                    o_ap = bass.AP(tensor=woh.tensor, offset=woh.offset + lo * S,
                                   ap=[woh.ap[0], [S * CPS, p.SUBS], [S + R, CPS],
                                       [1, R]])
                    i0 = bass.AP(tensor=m1t.tensor, offset=m1t.offset + lo * R,
                                 ap=[m1t.ap[0], [R * CPS, p.SUBS], [R, CPS], [1, R]])
                    i1 = bass.AP(tensor=wv.tensor, offset=wv.offset + lo,
                                 ap=[wv.ap[0], [CPS, p.SUBS], [1, CPS], [0, R]])
                    nc.vector.tensor_mul(o_ap, i0, i1)

                    pst = ps1pool.tile([p.P2R, FA], FP, tag="ps1", name=f"ps1_{t}")
                    for q in range(CPT):
                        sub, idx = q // CPS, q % CPS
                        cl = lo + q
                        nc.tensor.matmul(
                            pst[32 * sub: 32 * sub + S, :],
                            woh[:, cl, :], xt[:, cl, :],
                            start=(idx == 0), stop=(idx == CPS - 1),
                        )
                    p2b = sb.tile([p.P2R, FA], FP, tag="p2b", name=f"p2b_{t}")
                    nc.scalar.copy(p2b[:], pst[:])
                    nc.tensor.matmul(
                        psw[:], m2w[:, tl * WIN:(tl + 1) * WIN], p2b[:],
                        start=(tl == 0), stop=(tl == p.TPW - 1),
                    )
                nc.scalar.copy(uts[:WIN, v * FA:(v + 1) * FA], psw[:])

        # ---- global Z ----------------------------------------------------
        with tc.tile_pool(name="ps2", bufs=1, space="PSUM") as ps2pool, \
             tc.tile_pool(name="pse", bufs=2, space="PSUM") as psepool, \
             tc.tile_pool(name="sbe", bufs=2) as sbe, \
             tc.tile_pool(name="dram", bufs=2, space="DRAM") as dram:
            # Z_local = sum over windows/graphs of W row (col F of each window)
            zcol = sbe.tile([128, 1], FP, tag="zcol")
            wcols = bass.AP(tensor=uts.tensor, offset=uts.offset + F,
                            ap=[[uts.ap[0][0], WIN], [FA, NWPC]])
            nc.vector.reduce_sum(zcol[:WIN, :], wcols, axis=mybir.AxisListType.X)
            ones = sbe.tile([128, 1], FP, tag="ones")
            nc.vector.memset(ones[:], 1.0)
            zps = ps2pool.tile([1, 1], FP, tag="zps")
            nc.tensor.matmul(zps[:], zcol[:WIN, :], ones[:WIN, :],
                             start=True, stop=True)
            zpad = sbe.tile([1, 8], FP, tag="zpad")
            nc.vector.memset(zpad[:], 0.0)
            nc.vector.tensor_copy(zpad[:, 0:1], zps[:])
            zin = dram.tile([1, 8], FP)
            zout = dram.tile([1, 8], FP)
            nc.sync.dma_start(zin[:], zpad[:])
            nc.gpsimd.collective_compute(
                "AllReduce", mybir.AluOpType.add,
                replica_groups=[list(range(NCORES))],
                ins=[zin.opt()], outs=[zout.opt()],
            )
            zsum = sbe.tile([1, 8], FP, tag="zsum")
            nc.sync.dma_start(zsum[:], zout[:])
            zinv = sbe.tile([1, 1], FP, tag="zinv")
            nc.vector.reciprocal(zinv[:], zsum[:, 0:1])
            # broadcast 1/Z to FA partitions via rank-1 matmul
            ones_row = sbe.tile([1, FA], FP, tag="ones_row")
            nc.vector.memset(ones_row[:], 1.0)
            zbps = ps2pool.tile([FA, 1], FP, tag="zbps")
            nc.tensor.matmul(zbps[:], ones_row[:], zinv[:], start=True, stop=True)
            zinv_a = sbe.tile([FA, 1], FP, tag="zinv_a")
            nc.scalar.copy(zinv_a[:], zbps[:])

            # ---- transpose uT windows into uu [FA, gpc] ---------------------
            for v in range(NWPC):
                tps = psepool.tile([FA, WIN], FP, tag="tps")
                nc.tensor.transpose(tps[:], uts[:WIN, v * FA:(v + 1) * FA],
                                    ident_t[:WIN, :WIN])
                nc.scalar.copy(uu[:, v * WIN:(v + 1) * WIN], tps[:])

            # scale by 1/Z
            nc.vector.tensor_scalar_mul(uu[:], uu[:], zinv_a[:])

            # ---- heads -----------------------------------------------------
            outsb = persist.tile([OSUM, gpc], FP)
            NHALF = -(-gpc // 512)
            for nh in range(NHALF):
                g0 = nh * 512
                gn = min(512, gpc - g0)
                ps2 = psepool.tile([OSUM, gn], FP, tag="ps2h", bufs=1)
                s1s = []
                for h in range(4):
                    ps1h = psepool.tile([H, gn], FP, tag="ps1h")
                    nc.tensor.matmul(ps1h[:], aab_t[:, h * H:(h + 1) * H],
                                     uu[:, g0:g0 + gn], start=True, stop=False)
                    nc.tensor.matmul(ps1h[:], w1b_t[:, h * H:(h + 1) * H],
                                     sgt_t[:, g0:g0 + gn], start=False, stop=True)
                    s1 = sbe.tile([H, gn], FP, tag="s1")
                    nc.scalar.activation(s1[:], ps1h[:],
                                         mybir.ActivationFunctionType.Relu,
                                         bias=b1_t[:, h:h + 1])
                    s1s.append(s1)
                for h in range(4):
                    nc.tensor.matmul(ps2[:], w2p_t[:, h * OSUM:(h + 1) * OSUM],
                                     s1s[h][:], start=(h == 0), stop=(h == 3))
                nc.vector.tensor_scalar_add(outsb[:, g0:g0 + gn], ps2[:], b2_t[:])
            nc.sync.dma_start(out[:], outsb[:])

    nc.compile()
    return nc


# --------------------------------------------------------------------------
# host-side data prep
# --------------------------------------------------------------------------
def _prep_inputs(p, x, batch, space_group, weights):
    (w_proj, b_proj, w_att, b_att, sg_table,
     we1, be1, we2, be2, ws1, bs1, ws2, bs2,
     wc1, bc1, wc2, bc2, wm1, bm1, wm2, bm2) = weights

    wp = w_proj.astype(np.float64)
    bp = b_proj.astype(np.float64)
    v = (wp @ w_att.astype(np.float64)).ravel()
    vrep = np.broadcast_to(v.astype(np.float32), (128, F)).copy()
    ident = np.eye(128, dtype=np.float32)

    h1s = [(we1, be1), (ws1, bs1), (wc1, bc1), (wm1, bm1)]
    h2s = [(we2, be2), (ws2, bs2), (wc2, bc2), (wm2, bm2)]
    aab = np.zeros((FA, 4 * H), np.float32)
    w1b = np.zeros((H, 4 * H), np.float32)
    b1 = np.zeros((H, 4), np.float32)
    w2p = np.zeros((H, 4 * OSUM), np.float32)
    b2 = np.zeros((OSUM, 1), np.float32)
    for h, ((w1, bb1), (w2, bb2)) in enumerate(zip(h1s, h2s)):
        w1 = w1.astype(np.float64)
        aab[:F, h * H:(h + 1) * H] = (wp @ w1[:H]).astype(np.float32)
        aab[F, h * H:(h + 1) * H] = (bp @ w1[:H]).astype(np.float32)
        w1b[:, h * H:(h + 1) * H] = w1[H:].astype(np.float32)
        b1[:, h] = bb1.astype(np.float32)
        od, off = OD[h], ODOFF[h]
        w2p[:, h * OSUM + off: h * OSUM + off + od] = w2.astype(np.float32)
        b2[off:off + od, 0] = bb2.astype(np.float32)

    sg = sg_table.astype(np.float32)[np.asarray(space_group)]   # [G, H]

    in_maps = []
    for s in range(NCORES):
        xr = np.zeros((p.NCH, 128, FA), np.float32)
        xr[:, :, F] = 1.0
        m1 = np.zeros((p.NCH, 128, p.R), np.float32)
        for vwin in range(p.NWPC):
            wg = s * p.NWPC + vwin
            lo, hi = p.wstart[wg], p.wend[wg]
            n = hi - lo
            c0 = vwin * p.NCH_W
            xw = x[lo:hi]
            full, rem = divmod(n, 128)
            blk = xr[c0:c0 + p.NCH_W]
            blk[:full, :, :F] = xw[:full * 128].reshape(full, 128, F)
            if rem:
                blk[full, :rem, :F] = xw[full * 128:]
            b = (batch[lo:hi] - wg * p.WIN).astype(np.int64)
            ii = np.arange(n)
            cc = c0 + ii // 128
            pp = ii % 128
            rr = b - p.a[s, cc]
            m1[cc, pp, rr] = 1.0
        m2 = np.zeros((p.NB2, p.P2R, p.WIN), np.float32)
        for t in range(p.NB2):
            for c in range(t * p.CPT, (t + 1) * p.CPT):
                if p.span[s, c] == 0:
                    continue
                q = c % p.CPT
                base = 32 * (q // p.CPS) + p.R * (q % p.CPS)
                ac = p.a[s, c]
                for r in range(p.span[s, c]):
                    m2[t, base + r, ac + r] = 1.0
        in_maps.append({
            "xr": np.ascontiguousarray(xr.transpose(1, 0, 2)).reshape(128, p.NCH * FA),
            "m1": np.ascontiguousarray(m1.transpose(1, 0, 2)).reshape(128, p.NCH * p.R),
            "m2": np.ascontiguousarray(m2.transpose(1, 0, 2)).reshape(p.P2R, p.NB2 * p.WIN),
            "sgt": np.ascontiguousarray(sg[s * p.gpc:(s + 1) * p.gpc].T),
            "vrep": vrep, "ident": ident, "aab": aab, "w1b": w1b,
            "w2p": w2p, "b1": b1, "b2": b2,
        })
    return in_maps


_CACHE = {}


def _get_compiled(p):
    nc = _CACHE.get(p.key)
    if nc is None:
        nc = _build(p)
        _CACHE[p.key] = nc
    return nc


def kernel(x, batch, space_group, w_proj, b_proj, w_att, b_att, sg_table,
           we1, be1, we2, be2, ws1, bs1, ws2, bs2,
           wc1, bc1, wc2, bc2, wm1, bm1, wm2, bm2,
           _run_hw=None, _want_results=False):
    x = np.asarray(x, np.float32)
    batch = np.asarray(batch)
    n_nodes = x.shape[0]
    n_graphs = int(np.asarray(space_group).shape[0])
    p = _make_plan(batch, n_nodes, n_graphs)
    nc = _get_compiled(p)
    weights = (w_proj, b_proj, w_att, b_att, sg_table,
               we1, be1, we2, be2, ws1, bs1, ws2, bs2,
               wc1, bc1, wc2, bc2, wm1, bm1, wm2, bm2)
    in_maps = _prep_inputs(p, x, batch, space_group,
                           tuple(np.asarray(w, np.float32) for w in weights))
    run = _run_hw or (lambda nc_, im_: bass_utils.run_bass_kernel_spmd(
        nc_, im_, core_ids=list(range(NCORES))))
    res = run(nc, in_maps)
    outs = [res.results[s]["out"] for s in range(NCORES)]
    full = np.concatenate(outs, axis=1).T          # [G, 77]
    energy = np.ascontiguousarray(full[:, 0:64])
    stability = np.ascontiguousarray(full[:, 64:67])
    crystal = np.ascontiguousarray(full[:, 67:74])
    material = np.ascontiguousarray(full[:, 74:77])
    ret = (energy, stability, crystal, material)
    if _want_results:
        return ret, res
    return ret


# revision 13
# speedup vs baseline: 1.4196x; 1.4196x over previous
"""M3GNet multi-task head kernel for 8 Trainium2 NeuronCores.

Math restructuring (exactly equivalent to the reference up to fp reassociation):
  logits_i = x_i @ v + c0,  v = w_proj @ w_att, c0 = b_proj@w_att + b_att.
  softmax over all nodes: alpha_i = exp(logits_i)/Z  (c0 and the max-subtraction
  cancel in the ratio; |x@v| < ~0.3 so exp is numerically safe).
  pooled[g] = sum_{i in g} alpha_i * h_i
            = ( (sum_{i in g} w_i x_i) @ w_proj + (sum_{i in g} w_i) b_proj ) / Z
  with w_i = exp(x_i @ v).  So the device only needs weighted segment sums of x
  (u[g] in R^64) plus the weight sums W[g]; h is never materialized.
  Head layer 1:  relu(pooled @ w1_top + sg @ w1_bot + b1)
     pooled @ w1_top = (1/Z) * uhat @ (w_proj @ w1_top  stacked with b_proj@w1_top)
  so per head we precompute A_h = [[w_proj @ w1_top_h], [b_proj @ w1_top_h]] (65x128).

Distribution: graph-aligned data parallel.  Core s owns graphs
[s*G/8, (s+1)*G/8); since `batch` is sorted its nodes form a contiguous range.
Only the scalar Z = sum_i w_i needs an AllReduce.

SPMD uniformity: one program runs on all 8 cores, so no instruction operand may
depend on per-core data.  All data-dependent structure is carried in DMA'd
mask *content*:
  Level 1: each 128-node chunk is reduced to <=R "slots" via a narrow one-hot
    (R = max graphs spanned by any chunk).  Slot placement inside the psum tile
    is by *chunk index* (uniform): chunk c gets columns R*idx(c) of a zero-padded
    [128, S] lhsT slab, matmul out = psum[32*sub : 32*sub+S] (base in {0,32,64}).
  Level 2: a [96, WIN] 0/1 matrix (host-built per core) maps the 96 slot rows of
    each psum tile to the 128 graphs of the owning window(s); accumulated into a
    per-window [WIN, 65] psum tile at base 0.  All offsets uniform.
Host pads every window (128 consecutive graphs) to a common node count so all
loop trip counts are identical across cores.
"""

import contextlib

import ml_dtypes
import numpy as np

import concourse.bass as bass
import concourse.bacc as bacc
import concourse.tile as tile
from concourse import mybir
from concourse import bass_utils

NCORES = 8
F = 64          # x feature dim
FA = F + 1      # + ones column (gives the weight-sums W[g] for free)
H = 128         # hidden
OD = (64, 3, 7, 3)            # head output dims (energy, stability, crystal, material)
ODOFF = (0, 64, 67, 74)
OSUM = 77
SUBS = 3        # psum sub-blocks per level-1 tile (bases 0/32/64)

FP = mybir.dt.float32
BF = mybir.dt.bfloat16
FR = mybir.dt.float32r


# --------------------------------------------------------------------------
# planning (host, data-dependent but shape-uniform across cores)
# --------------------------------------------------------------------------
class Plan:
    pass


def _make_plan(batch, n_nodes, n_graphs):
    p = Plan()
    p.n_nodes = n_nodes
    p.n_graphs = n_graphs
    p.gpc = n_graphs // NCORES                     # graphs per core
    p.WIN = min(128, p.gpc)                        # graphs per window
    p.NWPC = p.gpc // p.WIN                        # windows per core
    nwg = NCORES * p.NWPC                          # global window count
    ws = np.searchsorted(batch, np.arange(nwg) * p.WIN, side="left")
    we = np.searchsorted(batch, (np.arange(nwg) + 1) * p.WIN, side="left")
    p.wstart, p.wend = ws, we
    wcnt = we - ws

    def spans_for(nch_w):
        nch = p.NWPC * nch_w
        a = np.zeros((NCORES, nch), np.int32)
        span = np.zeros((NCORES, nch), np.int32)
        for s in range(NCORES):
            for v in range(p.NWPC):
                wg = s * p.NWPC + v
                b = batch[ws[wg]:we[wg]] - (wg * p.WIN)
                nreal = len(b)
                for c in range(min(nch_w, -(-nreal // 128))):
                    lo, hi = c * 128, min((c + 1) * 128, nreal)
                    cc = v * nch_w + c
                    a[s, cc] = b[lo]
                    span[s, cc] = b[hi - 1] - b[lo] + 1
        return a, span

    # spans don't depend on tail padding; probe with minimal chunk count
    nch_w_raw = max(1, -(-int(wcnt.max()) // 128))
    _, span0 = spans_for(nch_w_raw)
    rmax = max(1, int(span0.max()))
    assert rmax <= 32, f"chunk spans {rmax} graphs > 32; layout assumption broken"
    p.R = 1 << (rmax - 1).bit_length()             # power of two -> divides 32
    p.CPS = 32 // p.R                              # chunks per 32-row sub-block
    p.S = 32                                       # lhsT slab width
    # pick sub-blocks per psum tile (1..3) minimizing window padding
    p.SUBS = min((-(-nch_w_raw // (p.CPS * s)) * (p.CPS * s), -s)
                 for s in (1, 2, 3))[1] * -1
    p.CPT = p.CPS * p.SUBS                         # chunks per level-1 psum tile
    p.P2R = 32 * p.SUBS                            # slot rows per psum tile
    # pad windows to whole level-1 psum tiles: no tile straddles a window
    p.NCH_W = -(-nch_w_raw // p.CPT) * p.CPT
    p.NWN = p.NCH_W * 128
    p.NCH = p.NWPC * p.NCH_W
    p.a, p.span = spans_for(p.NCH_W)
    p.NB2 = p.NCH // p.CPT                         # level-2 K-chunks (psum tiles)
    p.TPW = p.NCH_W // p.CPT                       # tiles per window
    p.key = (p.n_nodes, p.n_graphs, p.NWN, p.R, p.NWPC, p.WIN, p.SUBS)
    return p


# --------------------------------------------------------------------------
# device program
# --------------------------------------------------------------------------
def _build(p):
    nc = bacc.Bacc("TRN2", target_bir_lowering=False, debug=False,
                   num_devices=NCORES)
    WIN, NWPC, NCH_W, NCH = p.WIN, p.NWPC, p.NCH_W, p.NCH
    R, S, CPS, CPT, NB2 = p.R, p.S, p.CPS, p.CPT, p.NB2
    gpc = p.gpc

    xr = nc.dram_tensor("xr", [128, NCH * FA], BF, kind="ExternalInput").ap()
    m1 = nc.dram_tensor("m1", [128, NCH * R], BF, kind="ExternalInput").ap()
    m2 = nc.dram_tensor("m2", [p.P2R, NB2 * WIN], BF, kind="ExternalInput").ap()
    sgt = nc.dram_tensor("sgt", [H, gpc], BF, kind="ExternalInput").ap()
    vrep = nc.dram_tensor("vrep", [128, F], BF, kind="ExternalInput").ap()
    ident = nc.dram_tensor("ident", [128, 128], FP, kind="ExternalInput").ap()
    aab = nc.dram_tensor("aab", [FA, 4 * H], BF, kind="ExternalInput").ap()
    w1b = nc.dram_tensor("w1b", [H, 4 * H], BF, kind="ExternalInput").ap()
    w2p = nc.dram_tensor("w2p", [H, 4 * OSUM], BF, kind="ExternalInput").ap()
    b1 = nc.dram_tensor("b1", [H, 4], FP, kind="ExternalInput").ap()
    b2 = nc.dram_tensor("b2", [OSUM, 1], FP, kind="ExternalInput").ap()
    out = nc.dram_tensor("out", [OSUM, gpc], FP, kind="ExternalOutput").ap()

    xr3 = xr.rearrange("p (c f) -> p c f", c=NCH)
    m13 = m1.rearrange("p (c r) -> p c r", c=NCH)
    SL = 32                                    # logits slice width (chunks)

    with tile.TileContext(nc) as tc, contextlib.ExitStack() as ctx:
        const = ctx.enter_context(tc.tile_pool(name="const", bufs=1))
        sb = ctx.enter_context(tc.tile_pool(name="sb", bufs=2))
        sbw = ctx.enter_context(tc.tile_pool(name="sbw", bufs=3))
        persist = ctx.enter_context(tc.tile_pool(name="persist", bufs=1))

        vrep_t = const.tile([128, F], BF)
        nc.sync.dma_start(vrep_t[:], vrep[:])
        ident_t = const.tile([128, 128], FP)
        nc.sync.dma_start(ident_t[:], ident[:])
        sgt_t = const.tile([H, gpc], BF)
        nc.sync.dma_start(sgt_t[:], sgt[:])
        aab_t = const.tile([FA, 4 * H], BF)
        nc.sync.dma_start(aab_t[:], aab[:])
        w1b_t = const.tile([H, 4 * H], BF)
        nc.sync.dma_start(w1b_t[:], w1b[:])
        w2p_t = const.tile([H, 4 * OSUM], BF)
        nc.sync.dma_start(w2p_t[:], w2p[:])
        b1_t = const.tile([H, 4], FP)
        nc.sync.dma_start(b1_t[:], b1[:])
        b2_t = const.tile([OSUM, 1], FP)
        nc.sync.dma_start(b2_t[:], b2[:])

        uts = persist.tile([128, NWPC * FA], FP)     # per-window uT results
        uu = persist.tile([FA, gpc], BF)             # transposed (u | W) rows

        with tc.tile_pool(name="ps1", bufs=4, space="PSUM") as ps1pool, \
             tc.tile_pool(name="psw", bufs=3, space="PSUM") as pswpool:
            for v in range(NWPC):
                c0 = v * NCH_W
                xt = sbw.tile([128, NCH_W, FA], BF, tag="xt")
                nc.sync.dma_start(xt[:], xr3[:, c0:c0 + NCH_W, :])
                m1t = sbw.tile([128, NCH_W, R], BF, tag="m1t")
                nc.sync.dma_start(m1t[:], m13[:, c0:c0 + NCH_W, :])
                m2w = sbw.tile([p.P2R, p.TPW * WIN], BF, tag="m2w")
                nc.sync.dma_start(m2w[:], m2[:, v * p.TPW * WIN:(v + 1) * p.TPW * WIN])

                # logits -> exp weights, in SL-chunk slices
                lg = sb.tile([128, NCH_W], FP, tag="lg")
                for k in range(0, NCH_W, SL):
                    kn = min(SL, NCH_W - k)
                    prod = sb.tile([128, SL, F], BF, tag="prod", name=f"prod_{v}_{k}")
                    vb = bass.AP(tensor=vrep_t.tensor, offset=vrep_t.offset,
                                 ap=[vrep_t.ap[0], [0, kn], [1, F]])
                    nc.vector.tensor_mul(prod[:, :kn, :], xt[:, k:k + kn, 0:F], vb)
                    nc.vector.reduce_sum(lg[:, k:k + kn], prod[:, :kn, :],
                                         axis=mybir.AxisListType.X)
                wv = sb.tile([128, NCH_W], BF, tag="wv")
                nc.scalar.activation(wv[:], lg[:], mybir.ActivationFunctionType.Exp)

                # zero-padded slot slabs:  woh[p, c, R*idx(c)+r] = m1*w
                woh = sbw.tile([128, NCH_W, S], BF, tag="woh")
                nc.gpsimd.memset(woh[:], 0.0)
                psw = pswpool.tile([WIN, FA], FP, tag="psw", name=f"psw_{v}")
                for tl in range(p.TPW):                 # level-1 psum tiles
                    t = v * p.TPW + tl
                    lo = tl * CPT                       # chunk offset in window
                    o_ap = bass.AP(tensor=woh.tensor, offset=woh.offset + lo * S,
                                   ap=[woh.ap[0], [S * CPS, p.SUBS], [S + R, CPS],
                                       [1, R]])
                    i0 = bass.AP(tensor=m1t.tensor, offset=m1t.offset + lo * R,
                                 ap=[m1t.ap[0], [R * CPS, p.SUBS], [R, CPS], [1, R]])
                    i1 = bass.AP(tensor=wv.tensor, offset=wv.offset + lo,
                                 ap=[wv.ap[0], [CPS, p.SUBS], [1, CPS], [0, R]])
                    nc.vector.tensor_mul(o_ap, i0, i1)

                    pst = ps1pool.tile([p.P2R, FA], FP, tag="ps1", name=f"ps1_{t}")
                    for q in range(CPT):
                        sub, idx = q // CPS, q % CPS
                        cl = lo + q
                        nc.tensor.matmul(
                            pst[32 * sub: 32 * sub + S, :],
                            woh[:, cl, :], xt[:, cl, :],
                            start=(idx == 0), stop=(idx == CPS - 1),
                        )
                    p2b = sb.tile([p.P2R, FA], BF, tag="p2b", name=f"p2b_{t}")
                    nc.scalar.copy(p2b[:], pst[:])
                    nc.tensor.matmul(
                        psw[:], m2w[:, tl * WIN:(tl + 1) * WIN], p2b[:],
                        start=(tl == 0), stop=(tl == p.TPW - 1),
                    )
                nc.scalar.copy(uts[:WIN, v * FA:(v + 1) * FA], psw[:])

        # ---- global Z ----------------------------------------------------
        with tc.tile_pool(name="ps2", bufs=1, space="PSUM") as ps2pool, \
             tc.tile_pool(name="pse", bufs=2, space="PSUM") as psepool, \
             tc.tile_pool(name="sbe", bufs=2) as sbe, \
             tc.tile_pool(name="dram", bufs=2, space="DRAM") as dram:
            # Z_local = sum over windows/graphs of W row (col F of each window)
            zcol = sbe.tile([128, 1], FP, tag="zcol")
            wcols = bass.AP(tensor=uts.tensor, offset=uts.offset + F,
                            ap=[[uts.ap[0][0], WIN], [FA, NWPC]])
            nc.vector.reduce_sum(zcol[:WIN, :], wcols, axis=mybir.AxisListType.X)
            ones = sbe.tile([128, 1], FP, tag="ones")
            nc.vector.memset(ones[:], 1.0)
            zps = ps2pool.tile([1, 1], FP, tag="zps")
            nc.tensor.matmul(zps[:], zcol[:WIN, :], ones[:WIN, :],
                             start=True, stop=True)
            zpad = sbe.tile([1, 8], FP, tag="zpad")
            nc.vector.memset(zpad[:], 0.0)
            nc.vector.tensor_copy(zpad[:, 0:1], zps[:])
            zin = dram.tile([1, 8], FP)
            zout = dram.tile([1, 8], FP)
            nc.sync.dma_start(zin[:], zpad[:])
            nc.gpsimd.collective_compute(
                "AllReduce", mybir.AluOpType.add,
                replica_groups=[list(range(NCORES))],
                ins=[zin.opt()], outs=[zout.opt()],
            )
            # ---- transpose uT windows into uu [FA, gpc] ---------------------
            for v in range(NWPC):
                tps = psepool.tile([FA, WIN], FP, tag="tps")
                nc.tensor.transpose(tps[:], uts[:WIN, v * FA:(v + 1) * FA],
                                    ident_t[:WIN, :WIN])
                nc.scalar.copy(uu[:, v * WIN:(v + 1) * WIN], tps[:])

            zsum = sbe.tile([1, 8], FP, tag="zsum")
            nc.sync.dma_start(zsum[:], zout[:])
            zinv = sbe.tile([1, 1], FP, tag="zinv")
            nc.vector.reciprocal(zinv[:], zsum[:, 0:1])
            # broadcast 1/Z to FA partitions via rank-1 matmul
            ones_row = sbe.tile([1, FA], FP, tag="ones_row")
            nc.vector.memset(ones_row[:], 1.0)
            zbps = ps2pool.tile([FA, 1], FP, tag="zbps")
            nc.tensor.matmul(zbps[:], ones_row[:], zinv[:], start=True, stop=True)
            zinv_a = sbe.tile([FA, 1], FP, tag="zinv_a")
            nc.scalar.copy(zinv_a[:], zbps[:])

            # scale by 1/Z
            nc.vector.tensor_scalar_mul(uu[:], uu[:], zinv_a[:])

            # ---- heads -----------------------------------------------------
            outsb = persist.tile([OSUM, gpc], FP)
            NHALF = -(-gpc // 512)
            for nh in range(NHALF):
                g0 = nh * 512
                gn = min(512, gpc - g0)
                ps2 = psepool.tile([OSUM, gn], FP, tag="ps2h", bufs=1)
                s1s = []
                for h in range(4):
                    ps1h = psepool.tile([H, gn], FP, tag="ps1h")
                    nc.tensor.matmul(ps1h[:], aab_t[:, h * H:(h + 1) * H],
                                     uu[:, g0:g0 + gn], start=True, stop=False)
                    nc.tensor.matmul(ps1h[:], w1b_t[:, h * H:(h + 1) * H],
                                     sgt_t[:, g0:g0 + gn], start=False, stop=True)
                    s1 = sbe.tile([H, gn], BF, tag="s1")
                    nc.scalar.activation(s1[:], ps1h[:],
                                         mybir.ActivationFunctionType.Relu,
                                         bias=b1_t[:, h:h + 1])
                    s1s.append(s1)
                for h in range(4):
                    nc.tensor.matmul(ps2[:], w2p_t[:, h * OSUM:(h + 1) * OSUM],
                                     s1s[h][:], start=(h == 0), stop=(h == 3))
                nc.vector.tensor_scalar_add(outsb[:, g0:g0 + gn], ps2[:], b2_t[:])
            nc.sync.dma_start(out[:], outsb[:])

    nc.compile()
    return nc


# --------------------------------------------------------------------------
# host-side data prep
# --------------------------------------------------------------------------
def _prep_inputs(p, x, batch, space_group, weights):
    (w_proj, b_proj, w_att, b_att, sg_table,
     we1, be1, we2, be2, ws1, bs1, ws2, bs2,
     wc1, bc1, wc2, bc2, wm1, bm1, wm2, bm2) = weights

    wp = w_proj.astype(np.float64)
    bp = b_proj.astype(np.float64)
    v = (wp @ w_att.astype(np.float64)).ravel()
    vrep = np.broadcast_to(v.astype(ml_dtypes.bfloat16), (128, F)).copy()
    ident = np.eye(128, dtype=np.float32)

    h1s = [(we1, be1), (ws1, bs1), (wc1, bc1), (wm1, bm1)]
    h2s = [(we2, be2), (ws2, bs2), (wc2, bc2), (wm2, bm2)]
    aab = np.zeros((FA, 4 * H), ml_dtypes.bfloat16)
    w1b = np.zeros((H, 4 * H), ml_dtypes.bfloat16)
    b1 = np.zeros((H, 4), np.float32)
    w2p = np.zeros((H, 4 * OSUM), ml_dtypes.bfloat16)
    b2 = np.zeros((OSUM, 1), np.float32)
    for h, ((w1, bb1), (w2, bb2)) in enumerate(zip(h1s, h2s)):
        w1 = w1.astype(np.float64)
        aab[:F, h * H:(h + 1) * H] = (wp @ w1[:H]).astype(np.float32)
        aab[F, h * H:(h + 1) * H] = (bp @ w1[:H]).astype(np.float32)
        w1b[:, h * H:(h + 1) * H] = w1[H:].astype(np.float32)
        b1[:, h] = bb1.astype(np.float32)
        od, off = OD[h], ODOFF[h]
        w2p[:, h * OSUM + off: h * OSUM + off + od] = w2.astype(np.float32)
        b2[off:off + od, 0] = bb2.astype(np.float32)

    sg = sg_table.astype(ml_dtypes.bfloat16)[np.asarray(space_group)]   # [G, H]

    in_maps = []
    for s in range(NCORES):
        xr = np.zeros((p.NCH, 128, FA), ml_dtypes.bfloat16)
        xr[:, :, F] = 1.0
        m1 = np.zeros((p.NCH, 128, p.R), ml_dtypes.bfloat16)
        for vwin in range(p.NWPC):
            wg = s * p.NWPC + vwin
            lo, hi = p.wstart[wg], p.wend[wg]
            n = hi - lo
            c0 = vwin * p.NCH_W
            xw = x[lo:hi]
            full, rem = divmod(n, 128)
            blk = xr[c0:c0 + p.NCH_W]
            blk[:full, :, :F] = xw[:full * 128].reshape(full, 128, F)
            if rem:
                blk[full, :rem, :F] = xw[full * 128:]
            b = (batch[lo:hi] - wg * p.WIN).astype(np.int64)
            ii = np.arange(n)
            cc = c0 + ii // 128
            pp = ii % 128
            rr = b - p.a[s, cc]
            m1[cc, pp, rr] = 1.0
        m2 = np.zeros((p.NB2, p.P2R, p.WIN), ml_dtypes.bfloat16)
        for t in range(p.NB2):
            for c in range(t * p.CPT, (t + 1) * p.CPT):
                if p.span[s, c] == 0:
                    continue
                q = c % p.CPT
                base = 32 * (q // p.CPS) + p.R * (q % p.CPS)
                ac = p.a[s, c]
                for r in range(p.span[s, c]):
                    m2[t, base + r, ac + r] = 1.0
        in_maps.append({
            "xr": np.ascontiguousarray(xr.transpose(1, 0, 2)).reshape(128, p.NCH * FA),
            "m1": np.ascontiguousarray(m1.transpose(1, 0, 2)).reshape(128, p.NCH * p.R),
            "m2": np.ascontiguousarray(m2.transpose(1, 0, 2)).reshape(p.P2R, p.NB2 * p.WIN),
            "sgt": np.ascontiguousarray(sg[s * p.gpc:(s + 1) * p.gpc].T),
            "vrep": vrep, "ident": ident, "aab": aab, "w1b": w1b,
            "w2p": w2p, "b1": b1, "b2": b2,
        })
    return in_maps


_CACHE = {}


def _get_compiled(p):
    nc = _CACHE.get(p.key)
    if nc is None:
        nc = _build(p)
        _CACHE[p.key] = nc
    return nc


def kernel(x, batch, space_group, w_proj, b_proj, w_att, b_att, sg_table,
           we1, be1, we2, be2, ws1, bs1, ws2, bs2,
           wc1, bc1, wc2, bc2, wm1, bm1, wm2, bm2,
           _run_hw=None, _want_results=False):
    x = np.asarray(x, np.float32)
    batch = np.asarray(batch)
    n_nodes = x.shape[0]
    n_graphs = int(np.asarray(space_group).shape[0])
    p = _make_plan(batch, n_nodes, n_graphs)
    nc = _get_compiled(p)
    weights = (w_proj, b_proj, w_att, b_att, sg_table,
               we1, be1, we2, be2, ws1, bs1, ws2, bs2,
               wc1, bc1, wc2, bc2, wm1, bm1, wm2, bm2)
    in_maps = _prep_inputs(p, x, batch, space_group,
                           tuple(np.asarray(w, np.float32) for w in weights))
    run = _run_hw or (lambda nc_, im_: bass_utils.run_bass_kernel_spmd(
        nc_, im_, core_ids=list(range(NCORES))))
    res = run(nc, in_maps)
    outs = [res.results[s]["out"] for s in range(NCORES)]
    full = np.concatenate(outs, axis=1).T          # [G, 77]
    energy = np.ascontiguousarray(full[:, 0:64])
    stability = np.ascontiguousarray(full[:, 64:67])
    crystal = np.ascontiguousarray(full[:, 67:74])
    material = np.ascontiguousarray(full[:, 74:77])
    ret = (energy, stability, crystal, material)
    if _want_results:
        return ret, res
    return ret
